# revision 7
# baseline (speedup 1.0000x reference)
"""AutoRec forward kernel for Trainium2, 8-core SPMD.

Math (see reference):
    agg = segment_sum(r[:,None] * v[cols], rows, m)     # sparse (m,n) @ v
    h   = sigmoid(agg + mu)                             # (M, D)
    s   = sum(h[i] * w[j])                              # global scalar over E pairs
    out = s + b[j]                                      # (E,)

Device strategy (per core, users sharded):
  Each core owns RPC = 6272 rows (users). Both heavy stages are instances of
  one primitive: "gather rows from a replicated table, weight them, and
  segment-sum into a local per-row accumulator":
    phase 1: table=v (bf16), weights=r,     rows=ij[0], cols=ij[1] -> aggT
    phase 2: table=w (f32),  weights=1.0,   rows=i,     cols=j     -> aT
          (sum_e h[i_e] * w[j_e] = sum_u h[u] . A[u],  A[u] = sum_{i_e=u} w[j_e])
  The segment-sum runs on the tensor engine: for each chunk of 128 edges the
  gathered rows form the stationary operand [128e, 128d]; a one-hot matrix
  P[e, wrow] = weight_e * (local_row_e == wrow) built on DVE is the moving
  operand; psum accumulates aggT[d, wrow] over a 64-row window. Edges are
  pre-sorted by (table-half, window) on the host so windows are contiguous,
  and the static schedule (max chunk count per group across cores) is shared
  by all cores so one SPMD program serves all 8.
  Tables are split in two 25000-row halves because dma_gather indices are
  int16. Finally h = sigmoid(aggT + mu) in one ACT op and
  s_part = sum(hT * aT) reduced on DVE; the host sums the 8 partials and
  broadcasts s + b[j] (a trivial O(E) numpy gather).
"""

import math
from dataclasses import dataclass, field

import ml_dtypes
import numpy as np

# ---------------------------------------------------------------- config

CHUNK = 128  # edges per matmul (contraction = partition dim)
IDX_WRAP = 16  # dma_gather index wrap


@dataclass
class Cfg:
    M: int = 50000          # users (rows of spmm)
    dma_scratch: int = 16384  # SWDGE descriptor carveout (bytes)
    N: int = 50000          # items (table rows)
    D: int = 128            # feature dim (must be 128)
    ncores: int = 8
    rpc: int = 6272         # rows per core (multiple of window)
    window: int = 64        # psum row-window
    half: int = 25000       # table split (int16 index limit)
    call_chunks: int = 8    # chunks per dma_gather call (HW SWDGE ring caps ~1024 idxs/call)
    p1dt: str = "f16"       # value dtype of phase-1 gathers / one-hot
    p2dt: str = "f16"       # value dtype of phase-2 gathers / one-hot
    ttb: int = 512          # block size of the final fused mul-reduce

    @property
    def nwin(self):
        return self.rpc // self.window

    def __post_init__(self):
        assert self.rpc % self.window == 0
        assert self.rpc * self.ncores >= self.M
        assert self.N <= 2 * self.half and self.half <= 32767
        assert self.D == 128


FULL = Cfg()

# ---------------------------------------------------------------- host plan


@dataclass
class PhasePlan:
    groups: list          # [(hf, win, n_chunks)] in stream order (hf-major)
    calls: list           # [(hf, chunk_start, n_chunks)]
    total_chunks: int
    # per-core packed arrays
    idx_dram: list        # [ncores] int16 [128, total_chunks*8]
    wgt_dram: list        # [ncores] [128, total_chunks]
    rl_dram: list         # [ncores] [128, total_chunks]


def _wrap_idxs(ii: np.ndarray) -> np.ndarray:
    """[n] -> [128, n/16] wrapped (t -> (t%16, t//16)), replicated x8."""
    n = len(ii)
    a = ii.reshape(n // IDX_WRAP, IDX_WRAP).T
    return np.tile(a, (8, 1))


def plan_phase(cfg: Cfg, rows, cols, wgts) -> PhasePlan:
    rows = np.asarray(rows, np.int64)
    cols = np.asarray(cols, np.int64)
    wgts = np.asarray(wgts, np.float32)
    nwin, ncores, Wd = cfg.nwin, cfg.ncores, cfg.window

    core = rows // cfg.rpc
    local = rows - core * cfg.rpc
    win = local // Wd
    rl = (local - win * Wd).astype(np.float32)
    hf = (cols >= cfg.half).astype(np.int64)
    idx16 = (cols - hf * cfg.half).astype(np.int16)

    key = (core * 2 + hf) * nwin + win
    counts = np.bincount(key, minlength=ncores * 2 * nwin).reshape(ncores, 2, nwin)
    nch = -(-counts.max(axis=0) // CHUNK)  # [2, nwin] ceil
    groups = []
    gbase = np.zeros((2, nwin), np.int64)
    acc = 0
    for h in range(2):
        for w in range(nwin):
            n = int(nch[h, w])
            if n == 0:
                continue
            groups.append((h, w, n))
            gbase[h, w] = acc
            acc += n
    total_chunks = acc

    # gather calls: split each half's chunk-range into spans of call_chunks
    calls = []
    cur = 0
    for h in range(2):
        nh = int(nch[h][counts.max(axis=0)[h] > 0].sum()) if nwin else 0
        # recompute exactly: chunks of half h
        nh = sum(n for (hh, _, n) in groups if hh == h)
        off = cur
        while off < cur + nh:
            n = min(cfg.call_chunks, cur + nh - off)
            calls.append((h, off, n))
            off += n
        cur += nh
    assert cur == total_chunks

    idx_l, wgt_l, rl_l = [], [], []
    for c in range(ncores):
        mask = core == c
        eh, ew = hf[mask], win[mask]
        erl, ei, ewgt = rl[mask], idx16[mask], wgts[mask]
        order = np.lexsort((ew, eh))
        eh, ew, erl, ei, ewgt = (a[order] for a in (eh, ew, erl, ei, ewgt))
        gid = eh * nwin + ew
        # rank within each (hf,win) run of the sorted list
        if len(gid):
            first = np.r_[True, gid[1:] != gid[:-1]]
            run_start = np.maximum.accumulate(np.where(first, np.arange(len(gid)), 0))
            rank = np.arange(len(gid)) - run_start
        else:
            rank = np.zeros(0, np.int64)
        pos = gbase[eh, ew] * CHUNK + rank
        idx_full = np.zeros(total_chunks * CHUNK, np.int16)
        wgt_full = np.zeros(total_chunks * CHUNK, np.float32)
        rl_full = np.zeros(total_chunks * CHUNK, np.float32)
        idx_full[pos] = ei
        wgt_full[pos] = ewgt
        rl_full[pos] = erl

        # wrap idx per call
        parts = []
        for (_h, c0, n) in calls:
            parts.append(_wrap_idxs(idx_full[c0 * CHUNK:(c0 + n) * CHUNK]))
        idx_l.append(np.concatenate(parts, axis=1))
        wgt_l.append(wgt_full.reshape(-1, CHUNK).T.copy())
        rl_l.append(rl_full.reshape(-1, CHUNK).T.copy())

    return PhasePlan(groups, calls, total_chunks, idx_l, wgt_l, rl_l)


# ---------------------------------------------------------------- device build


def build_program(cfg: Cfg, ph1: PhasePlan, ph2: PhasePlan):
    import concourse.bacc as bacc
    import concourse.bass as bass
    import concourse.mybir as mybir
    import concourse.tile as tile

    f32 = mybir.dt.float32
    i16 = mybir.dt.int16
    DTMAP = {"f32": f32, "bf16": mybir.dt.bfloat16, "f16": mybir.dt.float16}
    p1dt, p2dt = DTMAP[cfg.p1dt], DTMAP[cfg.p2dt]
    P, Wd, RPC = 128, cfg.window, cfg.rpc
    n_hi = cfg.N - cfg.half

    nc = bacc.Bacc("TRN2", target_bir_lowering=False, debug=False,
                   dynamic_dma_scratch_size=cfg.dma_scratch)

    v_lo = nc.dram_tensor("v_lo", [cfg.half, cfg.D], p1dt, kind="ExternalInput")
    v_hi = nc.dram_tensor("v_hi", [n_hi, cfg.D], p1dt, kind="ExternalInput")
    w_lo = nc.dram_tensor("w_lo", [cfg.half, cfg.D], p2dt, kind="ExternalInput")
    w_hi = nc.dram_tensor("w_hi", [n_hi, cfg.D], p2dt, kind="ExternalInput")
    mu_c = nc.dram_tensor("mu_col", [P, 1], f32, kind="ExternalInput")

    def phase_params(tag, pl: PhasePlan):
        idx = nc.dram_tensor(f"idx{tag}", [P, pl.total_chunks * 8], i16,
                             kind="ExternalInput")
        wgt = nc.dram_tensor(f"wgt{tag}", [P, pl.total_chunks], f32,
                             kind="ExternalInput")
        rl = nc.dram_tensor(f"rl{tag}", [P, pl.total_chunks], f32,
                            kind="ExternalInput")
        return idx, wgt, rl

    idx1, wgt1, rl1 = phase_params(1, ph1)
    idx2, wgt2, rl2 = phase_params(2, ph2)
    s_out = nc.dram_tensor("s_out", [P, 1], f32, kind="ExternalOutput")

    with tile.TileContext(nc) as tc:
        with (
            tc.tile_pool(name="const", bufs=1) as cpool,
            tc.tile_pool(name="idxp", bufs=4) as ipool,
            tc.tile_pool(name="g1", bufs=3) as g1pool,
            tc.tile_pool(name="g2", bufs=3) as g2pool,
            tc.tile_pool(name="pp", bufs=8) as ppool,
            tc.tile_pool(name="ev", bufs=4) as evpool,
            tc.tile_pool(name="psum", bufs=8, space="PSUM") as pspool,
        ):
            # constants
            iota1 = cpool.tile([P, Wd], p1dt, tag="iota1")
            iota2 = cpool.tile([P, Wd], p2dt, tag="iota2")
            mu_t = cpool.tile([P, 1], f32, tag="mu")
            nc.gpsimd.iota(iota1[:], pattern=[[1, Wd]], base=0, channel_multiplier=0,
                           allow_small_or_imprecise_dtypes=True)
            nc.gpsimd.iota(iota2[:], pattern=[[1, Wd]], base=0, channel_multiplier=0,
                           allow_small_or_imprecise_dtypes=True)
            nc.sync.dma_start(mu_t[:], mu_c[:])

            acc1 = cpool.tile([P, RPC], f32, tag="acc1")
            acc2 = cpool.tile([P, RPC], f32, tag="acc2")
            nc.vector.memset(acc1[:], 0.0)
            nc.vector.memset(acc2[:], 0.0)

            wg1_t = cpool.tile([P, ph1.total_chunks], f32, tag="wg1")
            rl1_t = cpool.tile([P, ph1.total_chunks], f32, tag="rl1")
            wg2_t = cpool.tile([P, ph2.total_chunks], f32, tag="wg2")
            rl2_t = cpool.tile([P, ph2.total_chunks], f32, tag="rl2")
            nc.sync.dma_start(wg1_t[:], wgt1[:])
            nc.sync.dma_start(rl1_t[:], rl1[:])
            nc.sync.dma_start(wg2_t[:], wgt2[:])
            nc.sync.dma_start(rl2_t[:], rl2[:])

            def run_phase(pl: PhasePlan, tabs, idx_dram, wg_t, rl_t, acc, gpool,
                          pdt, io_t):
                # group bookkeeping: map chunk id -> (group, first?, last?)
                chunk_group = {}
                for g, (h, w, n) in enumerate(pl.groups):
                    base = sum(nn for (_, _, nn) in pl.groups[:g])
                    for k in range(n):
                        chunk_group[base + k] = (g, w, k == 0, k == n - 1)
                # consume calls in order, carrying the open psum group
                open_ps = None
                for (h, c0, n) in pl.calls:
                    it = ipool.tile([P, cfg.call_chunks * 8], i16, tag="idx")
                    nc.sync.dma_start(it[:, : n * 8],
                                      idx_dram[:, c0 * 8:(c0 + n) * 8])
                    gt = gpool.tile([P, cfg.call_chunks, cfg.D], pdt, tag="g")
                    nidx = n * CHUNK
                    nc.gpsimd.dma_gather(
                        gt[:, :n, :], tabs[h][:], it[:, : n * 8],
                        num_idxs=nidx, num_idxs_reg=nidx, elem_size=cfg.D,
                    )
                    for k in range(n):
                        cid = c0 + k
                        g, w, first, last = chunk_group[cid]
                        if first:
                            open_ps = pspool.tile([P, Wd], mybir.dt.float32,
                                                  tag="ps")
                        p_t = ppool.tile([P, Wd], pdt, tag="p")
                        nc.vector.tensor_scalar(
                            out=p_t[:], in0=io_t[:],
                            scalar1=rl_t[:, cid:cid + 1],
                            scalar2=wg_t[:, cid:cid + 1],
                            op0=mybir.AluOpType.is_equal,
                            op1=mybir.AluOpType.mult,
                        )
                        nc.tensor.matmul(open_ps[:], gt[:, k, :], p_t[:],
                                         start=first, stop=last)
                        if last:
                            sl = acc[:, w * Wd:(w + 1) * Wd]
                            nc.vector.tensor_tensor(
                                out=sl, in0=sl, in1=open_ps[:],
                                op=mybir.AluOpType.add)

            run_phase(ph1, (v_lo, v_hi), idx1, wg1_t, rl1_t, acc1, g1pool,
                      p1dt, iota1)
            run_phase(ph2, (w_lo, w_hi), idx2, wg2_t, rl2_t, acc2, g2pool,
                      p2dt, iota2)

            # h = sigmoid(aggT + mu)  (in place on acc1)
            nc.scalar.activation(acc1[:], acc1[:],
                                 mybir.ActivationFunctionType.Sigmoid,
                                 bias=mu_t[:, 0:1], scale=1.0)

            # s_part[p] = sum_d sum_u h[p,u]*A[p,u]  blockwise fused mul+reduce
            nblk = math.ceil(RPC / cfg.ttb)
            s_cols = cpool.tile([P, nblk], f32, tag="scols")
            for b in range(nblk):
                lo = b * cfg.ttb
                hi = min(RPC, lo + cfg.ttb)
                tmp = evpool.tile([P, cfg.ttb], f32, tag="tmp")
                nc.vector.tensor_tensor(
                    out=tmp[:, : hi - lo],
                    in0=acc1[:, lo:hi], in1=acc2[:, lo:hi],
                    op=mybir.AluOpType.mult)
                nc.vector.tensor_reduce(
                    s_cols[:, b:b + 1], tmp[:, : hi - lo],
                    axis=mybir.AxisListType.X, op=mybir.AluOpType.add)
            s_t = cpool.tile([P, 1], f32, tag="sfin")
            nc.vector.tensor_reduce(s_t[:], s_cols[:], axis=mybir.AxisListType.X,
                                    op=mybir.AluOpType.add)
            nc.sync.dma_start(s_out[:], s_t[:])

    nc.compile()
    return nc


# ---------------------------------------------------------------- host driver


NPDT = {"f32": np.float32, "bf16": ml_dtypes.bfloat16, "f16": np.float16}


def make_in_maps(cfg: Cfg, ph1: PhasePlan, ph2: PhasePlan, v, w, mu):
    p1np, p2np = NPDT[cfg.p1dt], NPDT[cfg.p2dt]
    v_lo = np.ascontiguousarray(v[:cfg.half].astype(p1np))
    v_hi = np.ascontiguousarray(v[cfg.half:].astype(p1np))
    w_lo = np.ascontiguousarray(w[:cfg.half].astype(p2np))
    w_hi = np.ascontiguousarray(w[cfg.half:].astype(p2np))
    mu_col = np.broadcast_to(mu.reshape(-1)[:, None], (128, 1)).astype(np.float32)
    mu_col = np.ascontiguousarray(mu_col)
    in_maps = []
    for c in range(cfg.ncores):
        in_maps.append({
            "v_lo": v_lo, "v_hi": v_hi, "w_lo": w_lo, "w_hi": w_hi,
            "mu_col": mu_col,
            "idx1": ph1.idx_dram[c], "wgt1": ph1.wgt_dram[c],
            "rl1": ph1.rl_dram[c],
            "idx2": ph2.idx_dram[c], "wgt2": ph2.wgt_dram[c],
            "rl2": ph2.rl_dram[c],
        })
    return in_maps


def prepare(cfg: Cfg, ij, r, i, j):
    ph1 = plan_phase(cfg, ij[0], ij[1], r)
    ph2 = plan_phase(cfg, i, j, np.ones(len(i), np.float32))
    return ph1, ph2


_prog_cache = {}


def kernel(ij, r, m, i, j, v, mu, w, b, cfg: Cfg = FULL, _return_parts=False,
           _run_kwargs=None):
    from concourse.bass_utils import run_bass_kernel_spmd

    ij = np.asarray(ij)
    r = np.asarray(r, np.float32)
    i = np.asarray(i)
    j = np.asarray(j)
    v = np.asarray(v, np.float32)
    w = np.asarray(w, np.float32)
    mu = np.asarray(mu, np.float32)
    b = np.asarray(b, np.float32)
    assert int(m) == cfg.M

    ph1, ph2 = prepare(cfg, ij, r, i, j)
    key = (cfg.M, cfg.N, ph1.total_chunks, ph2.total_chunks,
           tuple(ph1.groups), tuple(ph2.groups))
    if key not in _prog_cache:
        _prog_cache.clear()
        _prog_cache[key] = build_program(cfg, ph1, ph2)
    nc = _prog_cache[key]

    in_maps = make_in_maps(cfg, ph1, ph2, v, w, mu)
    res = run_bass_kernel_spmd(nc, in_maps, list(range(cfg.ncores)),
                               **(_run_kwargs or {}))
    parts = [res.results[c]["s_out"] for c in range(cfg.ncores)]
    s = np.float32(sum(np.asarray(p, np.float64).sum() for p in parts))
    out = s + b[j]
    if _return_parts:
        return out, res
    return out


# revision 10
# speedup vs baseline: 1.7924x; 1.7924x over previous
"""AutoRec forward kernel for Trainium2, 8-core SPMD.

Math (see reference):
    agg = segment_sum(r[:,None] * v[cols], rows, m)     # sparse (m,n) @ v
    h   = sigmoid(agg + mu)                             # (M, D)
    s   = sum(h[i] * w[j])                              # global scalar over E pairs
    out = s + b[j]                                      # (E,)

Device strategy (per core, users sharded):
  Each core owns RPC = 6272 rows (users). Both heavy stages are instances of
  one primitive: "gather rows from a replicated table, weight them, and
  segment-sum into a local per-row accumulator":
    phase 1: table=v (bf16), weights=r,     rows=ij[0], cols=ij[1] -> aggT
    phase 2: table=w (f32),  weights=1.0,   rows=i,     cols=j     -> aT
          (sum_e h[i_e] * w[j_e] = sum_u h[u] . A[u],  A[u] = sum_{i_e=u} w[j_e])
  The segment-sum runs on the tensor engine: for each chunk of 128 edges the
  gathered rows form the stationary operand [128e, 128d]; a one-hot matrix
  P[e, wrow] = weight_e * (local_row_e == wrow) built on DVE is the moving
  operand; psum accumulates aggT[d, wrow] over a 64-row window. Edges are
  pre-sorted by (table-half, window) on the host so windows are contiguous,
  and the static schedule (max chunk count per group across cores) is shared
  by all cores so one SPMD program serves all 8.
  Tables are split in two 25000-row halves because dma_gather indices are
  int16. Finally h = sigmoid(aggT + mu) in one ACT op and
  s_part = sum(hT * aT) reduced on DVE; the host sums the 8 partials and
  broadcasts s + b[j] (a trivial O(E) numpy gather).
"""

import math
from dataclasses import dataclass, field

import ml_dtypes
import numpy as np

# ---------------------------------------------------------------- config

CHUNK = 128  # edges per matmul (contraction = partition dim)
IDX_WRAP = 16  # dma_gather index wrap


@dataclass
class Cfg:
    M: int = 50000          # users (rows of spmm)
    dma_scratch: int = 16384  # SWDGE descriptor carveout (bytes)
    N: int = 50000          # items (table rows)
    D: int = 128            # feature dim (must be 128)
    ncores: int = 8
    rpc: int = 6272         # rows per core (multiple of window)
    window: int = 64        # psum row-window
    half: int = 25000       # table split (int16 index limit)
    call_chunks: int = 8    # chunks per dma_gather call (HW SWDGE ring caps ~1024 idxs/call)
    p1dt: str = "f16"       # value dtype of phase-1 gathers / one-hot
    p2dt: str = "f16"       # value dtype of phase-2 gathers / one-hot
    ttb: int = 512          # block size of the final fused mul-reduce
    queues: int = 4         # SWDGE queues to round-robin gather calls over
    host_p: bool = True     # precompute one-hot P on host, stream via HWDGE

    @property
    def nwin(self):
        return self.rpc // self.window

    def __post_init__(self):
        assert self.rpc % self.window == 0
        assert self.rpc * self.ncores >= self.M
        assert self.N <= 2 * self.half and self.half <= 32767
        assert self.D == 128


FULL = Cfg()

# ---------------------------------------------------------------- host plan


@dataclass
class PhasePlan:
    groups: list          # [(hf, win, n_chunks)] in stream order (hf-major)
    calls: list           # [(hf, chunk_start, n_chunks)]
    total_chunks: int
    # per-core packed arrays
    idx_dram: list        # [ncores] int16 [128, total_chunks*8]
    wgt_dram: list        # [ncores] [128, total_chunks]
    rl_dram: list         # [ncores] [128, total_chunks]
    p_dram: list = None   # [ncores] pdt [128, total_chunks*W] host one-hot


def _wrap_idxs(ii: np.ndarray) -> np.ndarray:
    """[n] -> [128, n/16] wrapped (t -> (t%16, t//16)), replicated x8."""
    n = len(ii)
    a = ii.reshape(n // IDX_WRAP, IDX_WRAP).T
    return np.tile(a, (8, 1))


def plan_phase(cfg: Cfg, rows, cols, wgts, pnp=None) -> PhasePlan:
    rows = np.asarray(rows, np.int64)
    cols = np.asarray(cols, np.int64)
    wgts = np.asarray(wgts, np.float32)
    nwin, ncores, Wd = cfg.nwin, cfg.ncores, cfg.window

    core = rows // cfg.rpc
    local = rows - core * cfg.rpc
    win = local // Wd
    rl = (local - win * Wd).astype(np.float32)
    hf = (cols >= cfg.half).astype(np.int64)
    idx16 = (cols - hf * cfg.half).astype(np.int16)

    key = (core * 2 + hf) * nwin + win
    counts = np.bincount(key, minlength=ncores * 2 * nwin).reshape(ncores, 2, nwin)
    nch = -(-counts.max(axis=0) // CHUNK)  # [2, nwin] ceil
    groups = []
    gbase = np.zeros((2, nwin), np.int64)
    acc = 0
    for h in range(2):
        for w in range(nwin):
            n = int(nch[h, w])
            if n == 0:
                continue
            groups.append((h, w, n))
            gbase[h, w] = acc
            acc += n
    total_chunks = acc

    # gather calls: split each half's chunk-range into spans of call_chunks
    calls = []
    cur = 0
    for h in range(2):
        nh = int(nch[h][counts.max(axis=0)[h] > 0].sum()) if nwin else 0
        # recompute exactly: chunks of half h
        nh = sum(n for (hh, _, n) in groups if hh == h)
        off = cur
        while off < cur + nh:
            n = min(cfg.call_chunks, cur + nh - off)
            calls.append((h, off, n))
            off += n
        cur += nh
    assert cur == total_chunks

    idx_l, wgt_l, rl_l, p_l = [], [], [], []
    for c in range(ncores):
        mask = core == c
        eh, ew = hf[mask], win[mask]
        erl, ei, ewgt = rl[mask], idx16[mask], wgts[mask]
        order = np.lexsort((ew, eh))
        eh, ew, erl, ei, ewgt = (a[order] for a in (eh, ew, erl, ei, ewgt))
        gid = eh * nwin + ew
        # rank within each (hf,win) run of the sorted list
        if len(gid):
            first = np.r_[True, gid[1:] != gid[:-1]]
            run_start = np.maximum.accumulate(np.where(first, np.arange(len(gid)), 0))
            rank = np.arange(len(gid)) - run_start
        else:
            rank = np.zeros(0, np.int64)
        pos = gbase[eh, ew] * CHUNK + rank
        idx_full = np.zeros(total_chunks * CHUNK, np.int16)
        wgt_full = np.zeros(total_chunks * CHUNK, np.float32)
        rl_full = np.zeros(total_chunks * CHUNK, np.float32)
        idx_full[pos] = ei
        wgt_full[pos] = ewgt
        rl_full[pos] = erl

        # wrap idx per call
        parts = []
        for (_h, c0, n) in calls:
            parts.append(_wrap_idxs(idx_full[c0 * CHUNK:(c0 + n) * CHUNK]))
        idx_l.append(np.concatenate(parts, axis=1))
        wgt_l.append(wgt_full.reshape(-1, CHUNK).T.copy())
        rl_l.append(rl_full.reshape(-1, CHUNK).T.copy())
        if pnp is not None:
            # host one-hot: P[p, chunk, x] = wgt * (rl == x), edge = chunk*128+p
            parr = np.zeros((CHUNK, total_chunks, Wd), pnp)
            epos = np.arange(total_chunks * CHUNK)
            parr[epos % CHUNK, epos // CHUNK, rl_full.astype(np.int64)] = wgt_full
            p_l.append(np.ascontiguousarray(parr.reshape(CHUNK, -1)))

    return PhasePlan(groups, calls, total_chunks, idx_l, wgt_l, rl_l,
                     p_l if pnp is not None else None)


# ---------------------------------------------------------------- device build


def build_program(cfg: Cfg, ph1: PhasePlan, ph2: PhasePlan):
    import concourse.bacc as bacc
    import concourse.bass as bass
    import concourse.mybir as mybir
    import concourse.tile as tile

    f32 = mybir.dt.float32
    i16 = mybir.dt.int16
    DTMAP = {"f32": f32, "bf16": mybir.dt.bfloat16, "f16": mybir.dt.float16}
    p1dt, p2dt = DTMAP[cfg.p1dt], DTMAP[cfg.p2dt]
    P, Wd, RPC = 128, cfg.window, cfg.rpc
    n_hi = cfg.N - cfg.half

    nc = bacc.Bacc("TRN2", target_bir_lowering=False, debug=False,
                   dynamic_dma_scratch_size=cfg.dma_scratch,
                   num_swdge_queues=cfg.queues)

    v_lo = nc.dram_tensor("v_lo", [cfg.half, cfg.D], p1dt, kind="ExternalInput")
    v_hi = nc.dram_tensor("v_hi", [n_hi, cfg.D], p1dt, kind="ExternalInput")
    w_lo = nc.dram_tensor("w_lo", [cfg.half, cfg.D], p2dt, kind="ExternalInput")
    w_hi = nc.dram_tensor("w_hi", [n_hi, cfg.D], p2dt, kind="ExternalInput")
    mu_c = nc.dram_tensor("mu_col", [P, 1], f32, kind="ExternalInput")

    def phase_params(tag, pl: PhasePlan, pdt):
        idx = nc.dram_tensor(f"idx{tag}", [P, pl.total_chunks * 8], i16,
                             kind="ExternalInput")
        if cfg.host_p:
            pd = nc.dram_tensor(f"p{tag}", [P, pl.total_chunks * Wd], pdt,
                                kind="ExternalInput")
            return idx, pd, None
        wgt = nc.dram_tensor(f"wgt{tag}", [P, pl.total_chunks], f32,
                             kind="ExternalInput")
        rl = nc.dram_tensor(f"rl{tag}", [P, pl.total_chunks], f32,
                            kind="ExternalInput")
        return idx, wgt, rl

    idx1, wgt1, rl1 = phase_params(1, ph1, p1dt)
    idx2, wgt2, rl2 = phase_params(2, ph2, p2dt)
    s_out = nc.dram_tensor("s_out", [P, 1], f32, kind="ExternalOutput")

    with tile.TileContext(nc) as tc:
        with (
            tc.tile_pool(name="const", bufs=1) as cpool,
            tc.tile_pool(name="idxp", bufs=4) as ipool,
            tc.tile_pool(name="g1", bufs=3) as g1pool,
            tc.tile_pool(name="g2", bufs=3) as g2pool,
            tc.tile_pool(name="pp", bufs=8) as ppool,
            tc.tile_pool(name="ev", bufs=4) as evpool,
            tc.tile_pool(name="psum", bufs=8, space="PSUM") as pspool,
        ):
            # constants
            mu_t = cpool.tile([P, 1], f32, tag="mu")
            nc.sync.dma_start(mu_t[:], mu_c[:])
            if not cfg.host_p:
                iota1 = cpool.tile([P, Wd], p1dt, tag="iota1")
                iota2 = cpool.tile([P, Wd], p2dt, tag="iota2")
                nc.gpsimd.iota(iota1[:], pattern=[[1, Wd]], base=0,
                               channel_multiplier=0,
                               allow_small_or_imprecise_dtypes=True)
                nc.gpsimd.iota(iota2[:], pattern=[[1, Wd]], base=0,
                               channel_multiplier=0,
                               allow_small_or_imprecise_dtypes=True)
            else:
                iota1 = iota2 = None

            acc1 = cpool.tile([P, RPC], f32, tag="acc1")
            acc2 = cpool.tile([P, RPC], f32, tag="acc2")
            nc.vector.memset(acc1[:], 0.0)
            nc.vector.memset(acc2[:], 0.0)

            if not cfg.host_p:
                wg1_t = cpool.tile([P, ph1.total_chunks], f32, tag="wg1")
                rl1_t = cpool.tile([P, ph1.total_chunks], f32, tag="rl1")
                wg2_t = cpool.tile([P, ph2.total_chunks], f32, tag="wg2")
                rl2_t = cpool.tile([P, ph2.total_chunks], f32, tag="rl2")
                nc.sync.dma_start(wg1_t[:], wgt1[:])
                nc.sync.dma_start(rl1_t[:], rl1[:])
                nc.sync.dma_start(wg2_t[:], wgt2[:])
                nc.sync.dma_start(rl2_t[:], rl2[:])
            else:
                wg1_t = rl1_t = wg2_t = rl2_t = None

            qcount = [0]

            def run_phase(pl: PhasePlan, tabs, idx_dram, p_dram, wg_t, rl_t,
                          acc, gpool, pdt, io_t):
                # group bookkeeping: map chunk id -> (group, first?, last?)
                chunk_group = {}
                for g, (h, w, n) in enumerate(pl.groups):
                    base = sum(nn for (_, _, nn) in pl.groups[:g])
                    for k in range(n):
                        chunk_group[base + k] = (g, w, k == 0, k == n - 1)
                # consume calls in order, carrying the open psum group
                open_ps = None
                for (h, c0, n) in pl.calls:
                    it = ipool.tile([P, cfg.call_chunks * 8], i16, tag="idx")
                    nc.sync.dma_start(it[:, : n * 8],
                                      idx_dram[:, c0 * 8:(c0 + n) * 8])
                    gt = gpool.tile([P, cfg.call_chunks, cfg.D], pdt, tag="g")
                    nidx = n * CHUNK
                    nc.gpsimd.dma_gather(
                        gt[:, :n, :], tabs[h][:], it[:, : n * 8],
                        num_idxs=nidx, num_idxs_reg=nidx, elem_size=cfg.D,
                        queue_num=qcount[0] % cfg.queues,
                    )
                    qcount[0] += 1
                    if cfg.host_p:
                        pc_t = ppool.tile([P, cfg.call_chunks * Wd], pdt,
                                          tag="pc")
                        nc.sync.dma_start(pc_t[:, : n * Wd],
                                          p_dram[:, c0 * Wd:(c0 + n) * Wd])
                    for k in range(n):
                        cid = c0 + k
                        g, w, first, last = chunk_group[cid]
                        if first:
                            open_ps = pspool.tile([P, Wd], mybir.dt.float32,
                                                  tag="ps")
                        if cfg.host_p:
                            p_ap = pc_t[:, k * Wd:(k + 1) * Wd]
                        else:
                            p_t = ppool.tile([P, Wd], pdt, tag="p")
                            nc.vector.tensor_scalar(
                                out=p_t[:], in0=io_t[:],
                                scalar1=rl_t[:, cid:cid + 1],
                                scalar2=wg_t[:, cid:cid + 1],
                                op0=mybir.AluOpType.is_equal,
                                op1=mybir.AluOpType.mult,
                            )
                            p_ap = p_t[:]
                        nc.tensor.matmul(open_ps[:], gt[:, k, :], p_ap,
                                         start=first, stop=last)
                        if last:
                            sl = acc[:, w * Wd:(w + 1) * Wd]
                            nc.vector.tensor_tensor(
                                out=sl, in0=sl, in1=open_ps[:],
                                op=mybir.AluOpType.add)

            run_phase(ph1, (v_lo, v_hi), idx1, wgt1, wg1_t, rl1_t, acc1,
                      g1pool, p1dt, iota1)
            run_phase(ph2, (w_lo, w_hi), idx2, wgt2, wg2_t, rl2_t, acc2,
                      g2pool, p2dt, iota2)

            # h = sigmoid(aggT + mu)  (in place on acc1)
            nc.scalar.activation(acc1[:], acc1[:],
                                 mybir.ActivationFunctionType.Sigmoid,
                                 bias=mu_t[:, 0:1], scale=1.0)

            # s_part[p] = sum_d sum_u h[p,u]*A[p,u]  blockwise fused mul+reduce
            nblk = math.ceil(RPC / cfg.ttb)
            s_cols = cpool.tile([P, nblk], f32, tag="scols")
            for b in range(nblk):
                lo = b * cfg.ttb
                hi = min(RPC, lo + cfg.ttb)
                tmp = evpool.tile([P, cfg.ttb], f32, tag="tmp")
                nc.vector.tensor_tensor(
                    out=tmp[:, : hi - lo],
                    in0=acc1[:, lo:hi], in1=acc2[:, lo:hi],
                    op=mybir.AluOpType.mult)
                nc.vector.tensor_reduce(
                    s_cols[:, b:b + 1], tmp[:, : hi - lo],
                    axis=mybir.AxisListType.X, op=mybir.AluOpType.add)
            s_t = cpool.tile([P, 1], f32, tag="sfin")
            nc.vector.tensor_reduce(s_t[:], s_cols[:], axis=mybir.AxisListType.X,
                                    op=mybir.AluOpType.add)
            nc.sync.dma_start(s_out[:], s_t[:])

    nc.compile()
    return nc


# ---------------------------------------------------------------- host driver


NPDT = {"f32": np.float32, "bf16": ml_dtypes.bfloat16, "f16": np.float16}


def make_in_maps(cfg: Cfg, ph1: PhasePlan, ph2: PhasePlan, v, w, mu):
    p1np, p2np = NPDT[cfg.p1dt], NPDT[cfg.p2dt]
    v_lo = np.ascontiguousarray(v[:cfg.half].astype(p1np))
    v_hi = np.ascontiguousarray(v[cfg.half:].astype(p1np))
    w_lo = np.ascontiguousarray(w[:cfg.half].astype(p2np))
    w_hi = np.ascontiguousarray(w[cfg.half:].astype(p2np))
    mu_col = np.broadcast_to(mu.reshape(-1)[:, None], (128, 1)).astype(np.float32)
    mu_col = np.ascontiguousarray(mu_col)
    in_maps = []
    for c in range(cfg.ncores):
        m = {
            "v_lo": v_lo, "v_hi": v_hi, "w_lo": w_lo, "w_hi": w_hi,
            "mu_col": mu_col,
            "idx1": ph1.idx_dram[c], "idx2": ph2.idx_dram[c],
        }
        if cfg.host_p:
            m["p1"] = ph1.p_dram[c]
            m["p2"] = ph2.p_dram[c]
        else:
            m.update({"wgt1": ph1.wgt_dram[c], "rl1": ph1.rl_dram[c],
                      "wgt2": ph2.wgt_dram[c], "rl2": ph2.rl_dram[c]})
        in_maps.append(m)
    return in_maps


def prepare(cfg: Cfg, ij, r, i, j):
    pnp1 = NPDT[cfg.p1dt] if cfg.host_p else None
    pnp2 = NPDT[cfg.p2dt] if cfg.host_p else None
    ph1 = plan_phase(cfg, ij[0], ij[1], r, pnp1)
    ph2 = plan_phase(cfg, i, j, np.ones(len(i), np.float32), pnp2)
    return ph1, ph2


_prog_cache = {}


def kernel(ij, r, m, i, j, v, mu, w, b, cfg: Cfg = FULL, _return_parts=False,
           _run_kwargs=None):
    from concourse.bass_utils import run_bass_kernel_spmd

    ij = np.asarray(ij)
    r = np.asarray(r, np.float32)
    i = np.asarray(i)
    j = np.asarray(j)
    v = np.asarray(v, np.float32)
    w = np.asarray(w, np.float32)
    mu = np.asarray(mu, np.float32)
    b = np.asarray(b, np.float32)
    assert int(m) == cfg.M

    ph1, ph2 = prepare(cfg, ij, r, i, j)
    key = (cfg.M, cfg.N, ph1.total_chunks, ph2.total_chunks,
           tuple(ph1.groups), tuple(ph2.groups))
    if key not in _prog_cache:
        _prog_cache.clear()
        _prog_cache[key] = build_program(cfg, ph1, ph2)
    nc = _prog_cache[key]

    in_maps = make_in_maps(cfg, ph1, ph2, v, w, mu)
    res = run_bass_kernel_spmd(nc, in_maps, list(range(cfg.ncores)),
                               **(_run_kwargs or {}))
    parts = [res.results[c]["s_out"] for c in range(cfg.ncores)]
    s = np.float32(sum(np.asarray(p, np.float64).sum() for p in parts))
    out = s + b[j]
    if _return_parts:
        return out, res
    return out


# revision 11
# speedup vs baseline: 2.0370x; 1.1365x over previous
"""AutoRec forward kernel for Trainium2, 8-core SPMD.

Math (see reference):
    agg = segment_sum(r[:,None] * v[cols], rows, m)     # sparse (m,n) @ v
    h   = sigmoid(agg + mu)                             # (M, D)
    s   = sum(h[i] * w[j])                              # global scalar over E pairs
    out = s + b[j]                                      # (E,)

Device strategy (per core, users sharded):
  Each core owns RPC = 6272 rows (users). Both heavy stages are instances of
  one primitive: "gather rows from a replicated table, weight them, and
  segment-sum into a local per-row accumulator":
    phase 1: table=v (bf16), weights=r,     rows=ij[0], cols=ij[1] -> aggT
    phase 2: table=w (f32),  weights=1.0,   rows=i,     cols=j     -> aT
          (sum_e h[i_e] * w[j_e] = sum_u h[u] . A[u],  A[u] = sum_{i_e=u} w[j_e])
  The segment-sum runs on the tensor engine: for each chunk of 128 edges the
  gathered rows form the stationary operand [128e, 128d]; a one-hot matrix
  P[e, wrow] = weight_e * (local_row_e == wrow) built on DVE is the moving
  operand; psum accumulates aggT[d, wrow] over a 64-row window. Edges are
  pre-sorted by (table-half, window) on the host so windows are contiguous,
  and the static schedule (max chunk count per group across cores) is shared
  by all cores so one SPMD program serves all 8.
  Tables are split in two 25000-row halves because dma_gather indices are
  int16. Finally h = sigmoid(aggT + mu) in one ACT op and
  s_part = sum(hT * aT) reduced on DVE; the host sums the 8 partials and
  broadcasts s + b[j] (a trivial O(E) numpy gather).
"""

import math
from dataclasses import dataclass, field

import ml_dtypes
import numpy as np

# ---------------------------------------------------------------- config

CHUNK = 128  # edges per matmul (contraction = partition dim)
IDX_WRAP = 16  # dma_gather index wrap


@dataclass
class Cfg:
    M: int = 50000          # users (rows of spmm)
    dma_scratch: int = 16384  # SWDGE descriptor carveout (bytes)
    N: int = 50000          # items (table rows)
    D: int = 128            # feature dim (must be 128)
    ncores: int = 8
    rpc: int = 6272         # rows per core (multiple of window)
    window: int = 128       # psum row-window
    half: int = 25000       # table split (int16 index limit)
    call_chunks: int = 8    # chunks per dma_gather call (HW SWDGE ring caps ~1024 idxs/call)
    p1dt: str = "f16"       # value dtype of phase-1 gathers / one-hot
    p2dt: str = "f16"       # value dtype of phase-2 gathers / one-hot
    ttb: int = 512          # block size of the final fused mul-reduce
    queues: int = 4         # SWDGE queues to round-robin gather calls over
    host_p: bool = True     # precompute one-hot P on host, stream via HWDGE

    @property
    def nwin(self):
        return self.rpc // self.window

    def __post_init__(self):
        assert self.rpc % self.window == 0
        assert self.rpc * self.ncores >= self.M
        assert self.N <= 2 * self.half and self.half <= 32767
        assert self.D == 128


FULL = Cfg()

# ---------------------------------------------------------------- host plan


@dataclass
class PhasePlan:
    groups: list          # [(hf, win, n_chunks)] in stream order (hf-major)
    calls: list           # [(hf, chunk_start, n_chunks)]
    total_chunks: int
    # per-core packed arrays
    idx_dram: list        # [ncores] int16 [128, total_chunks*8]
    wgt_dram: list        # [ncores] [128, total_chunks]
    rl_dram: list         # [ncores] [128, total_chunks]
    p_dram: list = None   # [ncores] pdt [128, total_chunks*W] host one-hot


def _wrap_idxs(ii: np.ndarray) -> np.ndarray:
    """[n] -> [128, n/16] wrapped (t -> (t%16, t//16)), replicated x8."""
    n = len(ii)
    a = ii.reshape(n // IDX_WRAP, IDX_WRAP).T
    return np.tile(a, (8, 1))


def plan_phase(cfg: Cfg, rows, cols, wgts, pnp=None) -> PhasePlan:
    rows = np.asarray(rows, np.int64)
    cols = np.asarray(cols, np.int64)
    wgts = np.asarray(wgts, np.float32)
    nwin, ncores, Wd = cfg.nwin, cfg.ncores, cfg.window

    core = rows // cfg.rpc
    local = rows - core * cfg.rpc
    win = local // Wd
    rl = (local - win * Wd).astype(np.float32)
    hf = (cols >= cfg.half).astype(np.int64)
    idx16 = (cols - hf * cfg.half).astype(np.int16)

    key = (core * 2 + hf) * nwin + win
    counts = np.bincount(key, minlength=ncores * 2 * nwin).reshape(ncores, 2, nwin)
    nch = -(-counts.max(axis=0) // CHUNK)  # [2, nwin] ceil
    groups = []
    gbase = np.zeros((2, nwin), np.int64)
    acc = 0
    for h in range(2):
        for w in range(nwin):
            n = int(nch[h, w])
            if n == 0:
                continue
            groups.append((h, w, n))
            gbase[h, w] = acc
            acc += n
    total_chunks = acc

    # gather calls: split each half's chunk-range into spans of call_chunks
    calls = []
    cur = 0
    for h in range(2):
        nh = int(nch[h][counts.max(axis=0)[h] > 0].sum()) if nwin else 0
        # recompute exactly: chunks of half h
        nh = sum(n for (hh, _, n) in groups if hh == h)
        off = cur
        while off < cur + nh:
            n = min(cfg.call_chunks, cur + nh - off)
            calls.append((h, off, n))
            off += n
        cur += nh
    assert cur == total_chunks

    idx_l, wgt_l, rl_l, p_l = [], [], [], []
    for c in range(ncores):
        mask = core == c
        eh, ew = hf[mask], win[mask]
        erl, ei, ewgt = rl[mask], idx16[mask], wgts[mask]
        order = np.lexsort((ew, eh))
        eh, ew, erl, ei, ewgt = (a[order] for a in (eh, ew, erl, ei, ewgt))
        gid = eh * nwin + ew
        # rank within each (hf,win) run of the sorted list
        if len(gid):
            first = np.r_[True, gid[1:] != gid[:-1]]
            run_start = np.maximum.accumulate(np.where(first, np.arange(len(gid)), 0))
            rank = np.arange(len(gid)) - run_start
        else:
            rank = np.zeros(0, np.int64)
        pos = gbase[eh, ew] * CHUNK + rank
        idx_full = np.zeros(total_chunks * CHUNK, np.int16)
        wgt_full = np.zeros(total_chunks * CHUNK, np.float32)
        rl_full = np.zeros(total_chunks * CHUNK, np.float32)
        idx_full[pos] = ei
        wgt_full[pos] = ewgt
        rl_full[pos] = erl

        # wrap idx per call
        parts = []
        for (_h, c0, n) in calls:
            parts.append(_wrap_idxs(idx_full[c0 * CHUNK:(c0 + n) * CHUNK]))
        idx_l.append(np.concatenate(parts, axis=1))
        wgt_l.append(wgt_full.reshape(-1, CHUNK).T.copy())
        rl_l.append(rl_full.reshape(-1, CHUNK).T.copy())
        if pnp is not None:
            # host one-hot: P[p, chunk, x] = wgt * (rl == x), edge = chunk*128+p
            parr = np.zeros((CHUNK, total_chunks, Wd), pnp)
            epos = np.arange(total_chunks * CHUNK)
            parr[epos % CHUNK, epos // CHUNK, rl_full.astype(np.int64)] = wgt_full
            p_l.append(np.ascontiguousarray(parr.reshape(CHUNK, -1)))

    return PhasePlan(groups, calls, total_chunks, idx_l, wgt_l, rl_l,
                     p_l if pnp is not None else None)


# ---------------------------------------------------------------- device build


def build_program(cfg: Cfg, ph1: PhasePlan, ph2: PhasePlan):
    import concourse.bacc as bacc
    import concourse.bass as bass
    import concourse.mybir as mybir
    import concourse.tile as tile

    f32 = mybir.dt.float32
    i16 = mybir.dt.int16
    DTMAP = {"f32": f32, "bf16": mybir.dt.bfloat16, "f16": mybir.dt.float16}
    p1dt, p2dt = DTMAP[cfg.p1dt], DTMAP[cfg.p2dt]
    P, Wd, RPC = 128, cfg.window, cfg.rpc
    n_hi = cfg.N - cfg.half

    nc = bacc.Bacc("TRN2", target_bir_lowering=False, debug=False,
                   dynamic_dma_scratch_size=cfg.dma_scratch,
                   num_swdge_queues=cfg.queues)

    v_lo = nc.dram_tensor("v_lo", [cfg.half, cfg.D], p1dt, kind="ExternalInput")
    v_hi = nc.dram_tensor("v_hi", [n_hi, cfg.D], p1dt, kind="ExternalInput")
    w_lo = nc.dram_tensor("w_lo", [cfg.half, cfg.D], p2dt, kind="ExternalInput")
    w_hi = nc.dram_tensor("w_hi", [n_hi, cfg.D], p2dt, kind="ExternalInput")
    mu_c = nc.dram_tensor("mu_col", [P, 1], f32, kind="ExternalInput")

    def phase_params(tag, pl: PhasePlan, pdt):
        idx = nc.dram_tensor(f"idx{tag}", [P, pl.total_chunks * 8], i16,
                             kind="ExternalInput")
        if cfg.host_p:
            pd = nc.dram_tensor(f"p{tag}", [P, pl.total_chunks * Wd], pdt,
                                kind="ExternalInput")
            return idx, pd, None
        wgt = nc.dram_tensor(f"wgt{tag}", [P, pl.total_chunks], f32,
                             kind="ExternalInput")
        rl = nc.dram_tensor(f"rl{tag}", [P, pl.total_chunks], f32,
                            kind="ExternalInput")
        return idx, wgt, rl

    idx1, wgt1, rl1 = phase_params(1, ph1, p1dt)
    idx2, wgt2, rl2 = phase_params(2, ph2, p2dt)
    s_out = nc.dram_tensor("s_out", [P, 1], f32, kind="ExternalOutput")

    with tile.TileContext(nc) as tc:
        with (
            tc.tile_pool(name="const", bufs=1) as cpool,
            tc.tile_pool(name="idxp", bufs=4) as ipool,
            tc.tile_pool(name="g1", bufs=3) as g1pool,
            tc.tile_pool(name="g2", bufs=3) as g2pool,
            tc.tile_pool(name="pp", bufs=8) as ppool,
            tc.tile_pool(name="ev", bufs=4) as evpool,
            tc.tile_pool(name="psum", bufs=8, space="PSUM") as pspool,
        ):
            # constants
            mu_t = cpool.tile([P, 1], f32, tag="mu")
            nc.sync.dma_start(mu_t[:], mu_c[:])
            if not cfg.host_p:
                iota1 = cpool.tile([P, Wd], p1dt, tag="iota1")
                iota2 = cpool.tile([P, Wd], p2dt, tag="iota2")
                nc.gpsimd.iota(iota1[:], pattern=[[1, Wd]], base=0,
                               channel_multiplier=0,
                               allow_small_or_imprecise_dtypes=True)
                nc.gpsimd.iota(iota2[:], pattern=[[1, Wd]], base=0,
                               channel_multiplier=0,
                               allow_small_or_imprecise_dtypes=True)
            else:
                iota1 = iota2 = None

            acc1 = cpool.tile([P, RPC], f32, tag="acc1")
            acc2 = cpool.tile([P, RPC], f32, tag="acc2")
            nc.vector.memset(acc1[:], 0.0)
            nc.vector.memset(acc2[:], 0.0)

            if not cfg.host_p:
                wg1_t = cpool.tile([P, ph1.total_chunks], f32, tag="wg1")
                rl1_t = cpool.tile([P, ph1.total_chunks], f32, tag="rl1")
                wg2_t = cpool.tile([P, ph2.total_chunks], f32, tag="wg2")
                rl2_t = cpool.tile([P, ph2.total_chunks], f32, tag="rl2")
                nc.sync.dma_start(wg1_t[:], wgt1[:])
                nc.sync.dma_start(rl1_t[:], rl1[:])
                nc.sync.dma_start(wg2_t[:], wgt2[:])
                nc.sync.dma_start(rl2_t[:], rl2[:])
            else:
                wg1_t = rl1_t = wg2_t = rl2_t = None

            qcount = [0]

            def run_phase(pl: PhasePlan, tabs, idx_dram, p_dram, wg_t, rl_t,
                          acc, gpool, pdt, io_t):
                # group bookkeeping: map chunk id -> (group, first?, last?)
                chunk_group = {}
                for g, (h, w, n) in enumerate(pl.groups):
                    base = sum(nn for (_, _, nn) in pl.groups[:g])
                    for k in range(n):
                        chunk_group[base + k] = (g, w, k == 0, k == n - 1)
                # consume calls in order, carrying the open psum group
                open_ps = None
                for (h, c0, n) in pl.calls:
                    it = ipool.tile([P, cfg.call_chunks * 8], i16, tag="idx")
                    nc.sync.dma_start(it[:, : n * 8],
                                      idx_dram[:, c0 * 8:(c0 + n) * 8])
                    gt = gpool.tile([P, cfg.call_chunks, cfg.D], pdt, tag="g")
                    nidx = n * CHUNK
                    nc.gpsimd.dma_gather(
                        gt[:, :n, :], tabs[h][:], it[:, : n * 8],
                        num_idxs=nidx, num_idxs_reg=nidx, elem_size=cfg.D,
                        queue_num=qcount[0] % cfg.queues,
                    )
                    qcount[0] += 1
                    if cfg.host_p:
                        pc_t = ppool.tile([P, cfg.call_chunks * Wd], pdt,
                                          tag="pc")
                        nc.sync.dma_start(pc_t[:, : n * Wd],
                                          p_dram[:, c0 * Wd:(c0 + n) * Wd])
                    for k in range(n):
                        cid = c0 + k
                        g, w, first, last = chunk_group[cid]
                        if first:
                            open_ps = pspool.tile([P, Wd], mybir.dt.float32,
                                                  tag="ps")
                        if cfg.host_p:
                            p_ap = pc_t[:, k * Wd:(k + 1) * Wd]
                        else:
                            p_t = ppool.tile([P, Wd], pdt, tag="p")
                            nc.vector.tensor_scalar(
                                out=p_t[:], in0=io_t[:],
                                scalar1=rl_t[:, cid:cid + 1],
                                scalar2=wg_t[:, cid:cid + 1],
                                op0=mybir.AluOpType.is_equal,
                                op1=mybir.AluOpType.mult,
                            )
                            p_ap = p_t[:]
                        nc.tensor.matmul(open_ps[:], gt[:, k, :], p_ap,
                                         start=first, stop=last)
                        if last:
                            sl = acc[:, w * Wd:(w + 1) * Wd]
                            nc.vector.tensor_tensor(
                                out=sl, in0=sl, in1=open_ps[:],
                                op=mybir.AluOpType.add)

            run_phase(ph1, (v_lo, v_hi), idx1, wgt1, wg1_t, rl1_t, acc1,
                      g1pool, p1dt, iota1)
            run_phase(ph2, (w_lo, w_hi), idx2, wgt2, wg2_t, rl2_t, acc2,
                      g2pool, p2dt, iota2)

            # h = sigmoid(aggT + mu)  (in place on acc1)
            nc.scalar.activation(acc1[:], acc1[:],
                                 mybir.ActivationFunctionType.Sigmoid,
                                 bias=mu_t[:, 0:1], scale=1.0)

            # s_part[p] = sum_d sum_u h[p,u]*A[p,u]  blockwise fused mul+reduce
            nblk = math.ceil(RPC / cfg.ttb)
            s_cols = cpool.tile([P, nblk], f32, tag="scols")
            for b in range(nblk):
                lo = b * cfg.ttb
                hi = min(RPC, lo + cfg.ttb)
                tmp = evpool.tile([P, cfg.ttb], f32, tag="tmp")
                nc.vector.tensor_tensor(
                    out=tmp[:, : hi - lo],
                    in0=acc1[:, lo:hi], in1=acc2[:, lo:hi],
                    op=mybir.AluOpType.mult)
                nc.vector.tensor_reduce(
                    s_cols[:, b:b + 1], tmp[:, : hi - lo],
                    axis=mybir.AxisListType.X, op=mybir.AluOpType.add)
            s_t = cpool.tile([P, 1], f32, tag="sfin")
            nc.vector.tensor_reduce(s_t[:], s_cols[:], axis=mybir.AxisListType.X,
                                    op=mybir.AluOpType.add)
            nc.sync.dma_start(s_out[:], s_t[:])

    nc.compile()
    return nc


# ---------------------------------------------------------------- host driver


NPDT = {"f32": np.float32, "bf16": ml_dtypes.bfloat16, "f16": np.float16}


def make_in_maps(cfg: Cfg, ph1: PhasePlan, ph2: PhasePlan, v, w, mu):
    p1np, p2np = NPDT[cfg.p1dt], NPDT[cfg.p2dt]
    v_lo = np.ascontiguousarray(v[:cfg.half].astype(p1np))
    v_hi = np.ascontiguousarray(v[cfg.half:].astype(p1np))
    w_lo = np.ascontiguousarray(w[:cfg.half].astype(p2np))
    w_hi = np.ascontiguousarray(w[cfg.half:].astype(p2np))
    mu_col = np.broadcast_to(mu.reshape(-1)[:, None], (128, 1)).astype(np.float32)
    mu_col = np.ascontiguousarray(mu_col)
    in_maps = []
    for c in range(cfg.ncores):
        m = {
            "v_lo": v_lo, "v_hi": v_hi, "w_lo": w_lo, "w_hi": w_hi,
            "mu_col": mu_col,
            "idx1": ph1.idx_dram[c], "idx2": ph2.idx_dram[c],
        }
        if cfg.host_p:
            m["p1"] = ph1.p_dram[c]
            m["p2"] = ph2.p_dram[c]
        else:
            m.update({"wgt1": ph1.wgt_dram[c], "rl1": ph1.rl_dram[c],
                      "wgt2": ph2.wgt_dram[c], "rl2": ph2.rl_dram[c]})
        in_maps.append(m)
    return in_maps


def prepare(cfg: Cfg, ij, r, i, j):
    pnp1 = NPDT[cfg.p1dt] if cfg.host_p else None
    pnp2 = NPDT[cfg.p2dt] if cfg.host_p else None
    ph1 = plan_phase(cfg, ij[0], ij[1], r, pnp1)
    ph2 = plan_phase(cfg, i, j, np.ones(len(i), np.float32), pnp2)
    return ph1, ph2


_prog_cache = {}


def kernel(ij, r, m, i, j, v, mu, w, b, cfg: Cfg = FULL, _return_parts=False,
           _run_kwargs=None):
    from concourse.bass_utils import run_bass_kernel_spmd

    ij = np.asarray(ij)
    r = np.asarray(r, np.float32)
    i = np.asarray(i)
    j = np.asarray(j)
    v = np.asarray(v, np.float32)
    w = np.asarray(w, np.float32)
    mu = np.asarray(mu, np.float32)
    b = np.asarray(b, np.float32)
    assert int(m) == cfg.M

    ph1, ph2 = prepare(cfg, ij, r, i, j)
    key = (cfg.M, cfg.N, ph1.total_chunks, ph2.total_chunks,
           tuple(ph1.groups), tuple(ph2.groups))
    if key not in _prog_cache:
        _prog_cache.clear()
        _prog_cache[key] = build_program(cfg, ph1, ph2)
    nc = _prog_cache[key]

    in_maps = make_in_maps(cfg, ph1, ph2, v, w, mu)
    res = run_bass_kernel_spmd(nc, in_maps, list(range(cfg.ncores)),
                               **(_run_kwargs or {}))
    parts = [res.results[c]["s_out"] for c in range(cfg.ncores)]
    s = np.float32(sum(np.asarray(p, np.float64).sum() for p in parts))
    out = s + b[j]
    if _return_parts:
        return out, res
    return out
